# revision 29
# baseline (speedup 1.0000x reference)
"""Trainium2 Bass kernel for conv-qkv linear-attention block.

Reference math (per sample b):
    q = conv3x3(x, wq) + bq ; k = conv3x3(x, wk) + bk ; v = conv3x3(x, wv) + bv
    kv[c] = sum_n k[c,n] * v[c,n]
    out = gamma * (q * kv[c]) + x

Strategy (v2 — dense tap-pair packing):
  - Data-parallel over batch: 32 samples -> 8 cores x 4 samples,
    processed as 2 pairs (A, B) per core.
  - Per-sample SBUF image buffer X_s = [128 partitions, 67, 66] bf16 where
    partitions 0-63 hold the zero-padded image (ch 0-63) and partitions
    64-127 hold the SAME image shifted down one row.  A single matmul rhs
    AP then presents TWO taps at once: lower half = tap (0,dx), upper
    half = tap (1,dx).  With dense 128x128 weights [2 taps x 64 in-ch ->
    q|k out-ch] the PE array runs at ~full utilization instead of the 50%
    of the old block-diagonal sample-pair scheme.
  - Per chunk of 512 pixels the conv work is 24 matmul instructions that
    execute in ~14 serial matmul-times (row/col tile_position concurrency
    packs the K=64 / M=64 leftovers into quarter-array slots):
      * 6 full qk-pair matmuls (3 per sample, K=128, M=128 = [q|k])
      * 6 qk "dy=2" singles (K=64, M=128), packed 2-concurrent
      * 6 v-pair matmuls (K=128, M=64), A/B packed col-concurrent
      * 6 v singles (K=64, M=64), packed 4-/2-concurrent
  - bf16 weights + images (full-rate PE, FWL weight loads, half DMA).
  - PSUM layouts psA=[qA|kA], psB=[kB|qB], psVA=[.|vA], psVB=[vB|.] are
    chosen so every downstream engine op is partition-aligned:
      * q drains (ScalarE, bias fused) land [qA|qB] in the pair layout
      * kv = sum((k~+bk)*v) fuses the k bias into the DVE
        scalar_tensor_tensor with accum_out, reading k~ from PSUM
      * only the final per-pair kv column needs a partition swap, done
        with two tiny SBUF->SBUF DMAs.
  - out = q*kv + x fused in one DVE scalar_tensor_tensor per chunk; the
    residual x is read back in bf16 from the padded image in DRAM.
  - gamma is folded into wq/bq on the host (exact algebra), so gamma=0
    gives q==0 exactly.
"""

import os

os.environ.setdefault("MYCRO_LOCAL_CACHE", "1")

try:  # pragma: no cover
    import antenv.axon_hooks  # noqa: F401
except Exception:
    os.environ["BASS_NEVER_TRACE"] = "1"

from contextlib import ExitStack

import numpy as np
import ml_dtypes

import concourse.bacc as bacc
import concourse.mybir as mybir
import concourse.tile as tile
from concourse.bass_utils import run_bass_kernel_spmd

B, C, H, W = 32, 64, 64, 64
NCORES = 8
BP = B // NCORES            # samples per core
PAIRS = BP // 2             # sample-pairs per core
HP, WP = H + 2, W + 2       # padded image (66 x 66)
HPD = H + 4                 # DRAM padded rows (68): extra zero rows for the
                            # shifted upper-half loads
WPD = W + 4                 # DRAM padded cols (68): extra zero col for the
                            # X2 upper-half load
X1ROWS = HP - 1             # X1 rows (65): upper half holds padded row t+1
X2ROWS = HP - 1             # X2 rows (65): lower half holds padded row t+2
RJ = 8                      # output rows per chunk
NCH = H // RJ               # chunks per image (8)
NF = RJ * W                 # moving free dim per matmul (512)

F32 = mybir.dt.float32
BF16 = mybir.dt.bfloat16
AF = mybir.ActivationFunctionType
ALU = mybir.AluOpType

LAST_RESULTS = None
_NC_CACHE = {}


def _build_nc(reps=1):
    nc = bacc.Bacc("TRN2", target_bir_lowering=False, debug=False)
    xpd = nc.dram_tensor("xpd", [BP, C, HPD, WPD], BF16, kind="ExternalInput")
    wqkp = nc.dram_tensor("wqkp", [128, 3, 2, 128], BF16, kind="ExternalInput")
    wqk2 = nc.dram_tensor("wqk2", [128, 2, 128], BF16, kind="ExternalInput")
    wqk1 = nc.dram_tensor("wqk1", [64, 2, 128], BF16, kind="ExternalInput")
    wvp = nc.dram_tensor("wvp", [128, 3, 64], BF16, kind="ExternalInput")
    wv2 = nc.dram_tensor("wv2", [128, 64], BF16, kind="ExternalInput")
    wv1 = nc.dram_tensor("wv1", [64, 64], BF16, kind="ExternalInput")
    bias = nc.dram_tensor("bias", [128, 3], F32, kind="ExternalInput")
    out = nc.dram_tensor("out", [BP, C, H, W], BF16, kind="ExternalOutput")

    xpd_ap = xpd.ap()
    out_ap = out.ap()

    with tile.TileContext(nc) as tc, ExitStack() as ctx:
        const_pool = ctx.enter_context(tc.tile_pool(name="const", bufs=1))
        x1_pool = ctx.enter_context(tc.tile_pool(name="x1", bufs=4))
        x2_pool = ctx.enter_context(tc.tile_pool(name="x2", bufs=4))
        xe_pool = ctx.enter_context(tc.tile_pool(name="xe", bufs=2))
        q_pool = ctx.enter_context(tc.tile_pool(name="qsb", bufs=2))
        vt_pool = ctx.enter_context(tc.tile_pool(name="vt", bufs=3))
        prod_pool = ctx.enter_context(tc.tile_pool(name="prod", bufs=3))
        red_pool = ctx.enter_context(tc.tile_pool(name="red", bufs=2))
        outp_pool = ctx.enter_context(tc.tile_pool(name="outp", bufs=2))
        psum_pool = ctx.enter_context(tc.tile_pool(name="psum", bufs=2, space="PSUM"))
        psv_pool = psum_pool

        wqkp_sb = const_pool.tile([128, 3, 2, 128], BF16)
        wqk2_sb = const_pool.tile([128, 2, 128], BF16)
        wqk1_sb = const_pool.tile([64, 2, 128], BF16)
        wvp_sb = const_pool.tile([128, 3, 64], BF16)
        wv2_sb = const_pool.tile([128, 64], BF16)
        wv1_sb = const_pool.tile([64, 64], BF16)
        b_sb = const_pool.tile([128, 3], F32)

        def _load_consts():
            nc.sync.dma_start(wqkp_sb[:], wqkp.ap())
            nc.sync.dma_start(wqk2_sb[:], wqk2.ap())
            nc.sync.dma_start(wqk1_sb[:], wqk1.ap())
            nc.sync.dma_start(wvp_sb[:], wvp.ap())
            nc.sync.dma_start(wv2_sb[:], wv2.ap())
            nc.sync.dma_start(wv1_sb[:], wv1.ap())
            nc.sync.dma_start(b_sb[:], bias.ap())

        def _body():
          # Hoist all image DMAs: HWDGE queues issue in order, so putting
          # both pairs' loads up front lets pair p+1's images stream while
          # pair p computes.
          # Per-sample images.  X1: lower half = padded rows 0..65, upper
          # half = padded rows 1..66 (one-row shift -> tap pairs
          # (0,c)+(1,c)).  X2: lower half = padded rows 2..66 (tap (2,c)),
          # upper half additionally shifted one col (tap (2,c+1)).
          # Tiles keep the full DRAM row width (WPD) so every load is one
          # contiguous run per partition; matmul APs slice columns for free.
          imgs, xes = [], []
          for p in range(PAIRS):
            x1_t, x2_t = [], []
            for s in range(2):
                src = xpd_ap[2 * p + s]
                src_flat = src.rearrange("c h w -> c (h w)")
                t1 = x1_pool.tile([128, X1ROWS, WPD], BF16, tag="x1")
                nc.sync.dma_start(t1[0:64], src[:, 0:X1ROWS, :])
                nc.scalar.dma_start(t1[64:128], src[:, 1:X1ROWS + 1, :])
                x1_t.append(t1)
                t2 = x2_pool.tile([128, X2ROWS, WPD], BF16, tag="x2")
                nc.gpsimd.dma_start(t2[0:64], src[:, 2:2 + X2ROWS, :])
                # one-col shift: contiguous flat run starting at (2,1)
                nc.gpsimd.dma_start(
                    t2[64:128],
                    src_flat[:, 2 * WPD + 1: 2 * WPD + 1 + X2ROWS * WPD]
                    .rearrange("c (h w) -> c h w", w=WPD),
                )
                x2_t.append(t2)
            imgs.append((x1_t[0], x1_t[1], x2_t[0], x2_t[1]))
            # residual x (bf16) in pair layout [A|B]; full-width rows so the
            # load is contiguous, interior selected in the consuming AP
            xe = xe_pool.tile([128, H, WPD], BF16)
            nc.gpsimd.dma_start(
                xe[:],
                xpd_ap[2 * p:2 * p + 2, :, 1:H + 1, :]
                .rearrange("b c h w -> (b c) h w"),
            )
            xes.append(xe)

          for p in range(PAIRS):
            X1A, X1B, X2A, X2B = imgs[p]
            xe = xes[p]

            q_sb = q_pool.tile([128, NCH, NF], F32)
            kvp = red_pool.tile([128, NCH], F32, tag="kvp")
            for j in range(NCH):
                rb = RJ * j
                psA = psum_pool.tile([128, NF], F32, tag="psA")
                psB = psum_pool.tile([128, NF], F32, tag="psB")
                # vA and vB share one bank: [vB | vA].  Each stream starts
                # its own partition range (has_written clears are
                # per-partition); skip_group_check silences the sim's
                # one-group-per-bank bookkeeping.
                psV = psv_pool.tile([128, NF], F32, tag="psV")
                mm = nc.tensor.matmul

                # qk matmuls, sample A then B: 3 X1 tap-pairs (K=128) +
                # 1 X2 tap-pair (K=128) + 1 X2 single (K=64), all rhs
                # base-partition 0, M=128 = [q|k] (A) / [k|q] (B)
                for v, X1, X2, ps in ((0, X1A, X2A, psA), (1, X1B, X2B, psB)):
                    for c in range(3):
                        mm(ps[:], wqkp_sb[:, c, v, :],
                           X1[:, rb:rb + RJ, c:c + W],
                           start=(c == 0), stop=False)
                    mm(ps[:], wqk2_sb[:, v, :],
                       X2[:, rb:rb + RJ, 0:W], start=False, stop=False)
                    mm(ps[:], wqk1_sb[:, v, :],
                       X2[0:64, rb:rb + RJ, 2:2 + W], start=False, stop=True)
                # v matmuls: same taps, M=64; A -> psVA[64:], B -> psVB[:64],
                # interleaved for col-group concurrency
                for c in range(3):
                    mm(psV[64:128], wvp_sb[:, c, :], X1A[:, rb:rb + RJ, c:c + W],
                       start=(c == 0), stop=False, skip_group_check=True)
                    mm(psV[0:64], wvp_sb[:, c, :], X1B[:, rb:rb + RJ, c:c + W],
                       start=(c == 0), stop=False, skip_group_check=True)
                mm(psV[64:128], wv2_sb[:], X2A[:, rb:rb + RJ, 0:W],
                   start=False, stop=False, skip_group_check=True)
                mm(psV[0:64], wv2_sb[:], X2B[:, rb:rb + RJ, 0:W],
                   start=False, stop=False, skip_group_check=True)
                mm(psV[64:128], wv1_sb[:], X2A[0:64, rb:rb + RJ, 2:2 + W],
                   start=False, stop=False, skip_group_check=True)
                mm(psV[0:64], wv1_sb[:], X2B[0:64, rb:rb + RJ, 2:2 + W],
                   start=False, stop=True, skip_group_check=True)

                # drains: q with fused bias (ScalarE), v with fused bias
                nc.scalar.activation(
                    q_sb[0:64, j, :], psA[0:64], AF.Identity,
                    bias=b_sb[0:64, 0:1])
                nc.scalar.activation(
                    q_sb[64:128, j, :], psB[64:128], AF.Identity,
                    bias=b_sb[64:128, 0:1])
                vt = vt_pool.tile([128, NF], F32, tag="vt")
                nc.scalar.activation(
                    vt[:], psV[:], AF.Identity, bias=b_sb[:, 2:3])
                # kv partial: (k~ + bk) * v summed over the chunk (DVE),
                # k~ read straight from PSUM
                prod = prod_pool.tile([128, NF], F32, tag="prod")
                nc.vector.scalar_tensor_tensor(
                    out=prod[64:128],
                    in0=psA[64:128],
                    scalar=b_sb[64:128, 1:2],
                    in1=vt[64:128],
                    op0=ALU.add,
                    op1=ALU.mult,
                    accum_out=kvp[64:128, j:j + 1],
                )
                nc.vector.scalar_tensor_tensor(
                    out=prod[0:64],
                    in0=psB[0:64],
                    scalar=b_sb[0:64, 1:2],
                    in1=vt[0:64],
                    op0=ALU.add,
                    op1=ALU.mult,
                    accum_out=kvp[0:64, j:j + 1],
                )

            # total kv per channel; kv = [kvB | kvA] -> swap to [kvA | kvB]
            kv = red_pool.tile([128, 1], F32, tag="kv")
            nc.vector.tensor_reduce(
                kv[:], kvp[:], axis=mybir.AxisListType.X, op=ALU.add
            )
            kvsw = red_pool.tile([128, 1], F32, tag="kvsw")
            # on the SWDGE ring so the image HWDGE rings stay pure prefetch
            nc.gpsimd.dma_start(kvsw[0:64], kv[64:128])
            nc.gpsimd.dma_start(kvsw[64:128], kv[0:64])

            # out = q * kv + x, one DVE op per chunk
            o_sb = outp_pool.tile([128, NCH, NF], BF16)
            for j in range(NCH):
                nc.vector.scalar_tensor_tensor(
                    out=o_sb[:, j, :].rearrange("p (a b) -> p a b", a=RJ),
                    in0=q_sb[:, j, :].rearrange("p (a b) -> p a b", a=RJ),
                    scalar=kvsw[:, 0:1],
                    in1=xe[:, RJ * j:RJ * j + RJ, 1:1 + W],
                    op0=ALU.mult,
                    op1=ALU.add,
                )
            nc.gpsimd.dma_start(
                out_ap[2 * p:2 * p + 2],
                o_sb[:],
            )

        if reps == 1:
            _load_consts()
            _body()
        else:
            from concourse.engine_type import EngineType

            _load_consts()
            with tc.For_i(0, reps, 1, hint_engines=(EngineType.PE,),
                          staggered_reset=True):
                _body()

    nc.compile()
    return nc


def _get_nc(reps=1):
    if reps not in _NC_CACHE:
        _NC_CACHE[reps] = _build_nc(reps)
    return _NC_CACHE[reps]


def _pack_weights(wq, bq, wk, bk, wv, bv, gamma):
    g = float(np.asarray(gamma, np.float32).reshape(-1)[0])
    wqg = np.asarray(wq, np.float32) * g
    wkf = np.asarray(wk, np.float32)
    wvf = np.asarray(wv, np.float32)
    bqg = np.asarray(bq, np.float32) * g
    bkf = np.asarray(bk, np.float32)
    bvf = np.asarray(bv, np.float32)

    def t(w, dy, dx):
        # lhsT block [in_ch, out_ch] for one tap
        return w[:, :, dy, dx].T

    # conv column order per sample variant: A = [q|k], B = [k|q]
    pairs_a = ((wqg, 0), (wkf, 64))
    pairs_b = ((wkf, 0), (wqg, 64))

    wqkp = np.zeros((128, 3, 2, 128), np.float32)
    for c in range(3):
        for v, cols in ((0, pairs_a), (1, pairs_b)):
            for w, o in cols:
                wqkp[0:64, c, v, o:o + 64] = t(w, 0, c)
                wqkp[64:128, c, v, o:o + 64] = t(w, 1, c)

    wqk2 = np.zeros((128, 2, 128), np.float32)
    wqk1 = np.zeros((64, 2, 128), np.float32)
    for v, cols in ((0, pairs_a), (1, pairs_b)):
        for w, o in cols:
            wqk2[0:64, v, o:o + 64] = t(w, 2, 0)
            wqk2[64:128, v, o:o + 64] = t(w, 2, 1)
            wqk1[:, v, o:o + 64] = t(w, 2, 2)

    wvp = np.zeros((128, 3, 64), np.float32)
    for c in range(3):
        wvp[0:64, c, :] = t(wvf, 0, c)
        wvp[64:128, c, :] = t(wvf, 1, c)
    wv2 = np.zeros((128, 64), np.float32)
    wv2[0:64, :] = t(wvf, 2, 0)
    wv2[64:128, :] = t(wvf, 2, 1)
    wv1 = t(wvf, 2, 2)

    bias = np.zeros((128, 3), np.float32)
    for c, b in enumerate((bqg, bkf, bvf)):
        bias[0:64, c] = b
        bias[64:128, c] = b

    bf = ml_dtypes.bfloat16
    return (wqkp.astype(bf), wqk2.astype(bf), wqk1.astype(bf),
            wvp.astype(bf), wv2.astype(bf), wv1.astype(bf), bias)


def _pack_inputs(x):
    xp = np.zeros((B, C, HPD, WPD), ml_dtypes.bfloat16)
    xp[:, :, 1:H + 1, 1:W + 1] = x.astype(ml_dtypes.bfloat16)
    return xp


def _in_maps(x, wq, bq, wk, bk, wv, bv, gamma):
    wqkp, wqk2, wqk1, wvp, wv2, wv1, bias = _pack_weights(
        wq, bq, wk, bk, wv, bv, gamma)
    xp = _pack_inputs(np.ascontiguousarray(np.asarray(x, np.float32)))
    return [
        {
            "xpd": xp[BP * i:BP * (i + 1)],
            "wqkp": wqkp,
            "wqk2": wqk2,
            "wqk1": wqk1,
            "wvp": wvp,
            "wv2": wv2,
            "wv1": wv1,
            "bias": bias,
        }
        for i in range(NCORES)
    ]


def kernel(x, wq, bq, wk, bk, wv, bv, gamma):
    x = np.ascontiguousarray(np.asarray(x, np.float32))
    assert x.shape == (B, C, H, W), x.shape
    in_maps = _in_maps(x, wq, bq, wk, bk, wv, bv, gamma)
    nc = _get_nc()
    res = run_bass_kernel_spmd(nc, in_maps, core_ids=list(range(NCORES)))
    global LAST_RESULTS
    LAST_RESULTS = res
    return np.concatenate(
        [np.asarray(res.results[i]["out"], np.float32) for i in range(NCORES)],
        axis=0,
    )


def time_kernel(inputs, reps_lo=512, reps_hi=8192, calls=3):
    """Estimate per-iteration HW exec time by differencing two on-device
    repeat-loop variants (call overhead and transfers cancel)."""
    import time as _time

    in_maps = _in_maps(
        inputs["x"], inputs["wq"], inputs["bq"], inputs["wk"], inputs["bk"],
        inputs["wv"], inputs["bv"], inputs["gamma"],
    )
    nc_lo, nc_hi = _get_nc(reps_lo), _get_nc(reps_hi)
    cores = list(range(NCORES))
    run_bass_kernel_spmd(nc_lo, in_maps, core_ids=cores)
    run_bass_kernel_spmd(nc_hi, in_maps, core_ids=cores)
    deltas = []
    walls = {}
    for _ in range(calls + 2):
        t0 = _time.time()
        run_bass_kernel_spmd(nc_lo, in_maps, core_ids=cores)
        t1 = _time.time()
        run_bass_kernel_spmd(nc_hi, in_maps, core_ids=cores)
        t2 = _time.time()
        walls[reps_lo] = min(walls.get(reps_lo, 1e9), t1 - t0)
        walls[reps_hi] = min(walls.get(reps_hi, 1e9), t2 - t1)
        deltas.append(((t2 - t1) - (t1 - t0)) / (reps_hi - reps_lo) * 1e9)
    deltas.sort()
    return deltas[len(deltas) // 2], walls


# revision 30
# speedup vs baseline: 1.0704x; 1.0704x over previous
"""Trainium2 Bass kernel for conv-qkv linear-attention block.

Reference math (per sample b):
    q = conv3x3(x, wq) + bq ; k = conv3x3(x, wk) + bk ; v = conv3x3(x, wv) + bv
    kv[c] = sum_n k[c,n] * v[c,n]
    out = gamma * (q * kv[c]) + x

Strategy (v2 — dense tap-pair packing):
  - Data-parallel over batch: 32 samples -> 8 cores x 4 samples,
    processed as 2 pairs (A, B) per core.
  - Per-sample SBUF image buffer X_s = [128 partitions, 67, 66] bf16 where
    partitions 0-63 hold the zero-padded image (ch 0-63) and partitions
    64-127 hold the SAME image shifted down one row.  A single matmul rhs
    AP then presents TWO taps at once: lower half = tap (0,dx), upper
    half = tap (1,dx).  With dense 128x128 weights [2 taps x 64 in-ch ->
    q|k out-ch] the PE array runs at ~full utilization instead of the 50%
    of the old block-diagonal sample-pair scheme.
  - Per chunk of 512 pixels the conv work is 24 matmul instructions that
    execute in ~14 serial matmul-times (row/col tile_position concurrency
    packs the K=64 / M=64 leftovers into quarter-array slots):
      * 6 full qk-pair matmuls (3 per sample, K=128, M=128 = [q|k])
      * 6 qk "dy=2" singles (K=64, M=128), packed 2-concurrent
      * 6 v-pair matmuls (K=128, M=64), A/B packed col-concurrent
      * 6 v singles (K=64, M=64), packed 4-/2-concurrent
  - bf16 weights + images (full-rate PE, FWL weight loads, half DMA).
  - PSUM layouts psA=[qA|kA], psB=[kB|qB], psVA=[.|vA], psVB=[vB|.] are
    chosen so every downstream engine op is partition-aligned:
      * q drains (ScalarE, bias fused) land [qA|qB] in the pair layout
      * kv = sum((k~+bk)*v) fuses the k bias into the DVE
        scalar_tensor_tensor with accum_out, reading k~ from PSUM
      * only the final per-pair kv column needs a partition swap, done
        with two tiny SBUF->SBUF DMAs.
  - out = q*kv + x fused in one DVE scalar_tensor_tensor per chunk; the
    residual x is read back in bf16 from the padded image in DRAM.
  - gamma is folded into wq/bq on the host (exact algebra), so gamma=0
    gives q==0 exactly.
"""

import os

os.environ.setdefault("MYCRO_LOCAL_CACHE", "1")

try:  # pragma: no cover
    import antenv.axon_hooks  # noqa: F401
except Exception:
    os.environ["BASS_NEVER_TRACE"] = "1"

from contextlib import ExitStack

import numpy as np
import ml_dtypes

import concourse.bacc as bacc
import concourse.mybir as mybir
import concourse.tile as tile
from concourse.bass_utils import run_bass_kernel_spmd

B, C, H, W = 32, 64, 64, 64
NCORES = 8
BP = B // NCORES            # samples per core
PAIRS = BP // 2             # sample-pairs per core
HP, WP = H + 2, W + 2       # padded image (66 x 66)
HPD = H + 4                 # DRAM padded rows (68): extra zero rows for the
                            # shifted upper-half loads
WPD = W + 4                 # DRAM padded cols (68): extra zero col for the
                            # X2 upper-half load
X1ROWS = HP - 1             # X1 rows (65): upper half holds padded row t+1
X2ROWS = HP - 1             # X2 rows (65): lower half holds padded row t+2
RJ = 8                      # output rows per chunk
NCH = H // RJ               # chunks per image (8)
NF = RJ * W                 # moving free dim per matmul (512)

F32 = mybir.dt.float32
BF16 = mybir.dt.bfloat16
AF = mybir.ActivationFunctionType
ALU = mybir.AluOpType

LAST_RESULTS = None
_NC_CACHE = {}


def _build_nc(reps=1):
    nc = bacc.Bacc("TRN2", target_bir_lowering=False, debug=False)
    xpd = nc.dram_tensor("xpd", [BP, C, HPD, WPD], BF16, kind="ExternalInput")
    wqkp = nc.dram_tensor("wqkp", [128, 3, 2, 128], BF16, kind="ExternalInput")
    wqk2 = nc.dram_tensor("wqk2", [128, 2, 128], BF16, kind="ExternalInput")
    wqk1 = nc.dram_tensor("wqk1", [64, 2, 128], BF16, kind="ExternalInput")
    wvp = nc.dram_tensor("wvp", [128, 3, 64], BF16, kind="ExternalInput")
    wv2 = nc.dram_tensor("wv2", [128, 64], BF16, kind="ExternalInput")
    wv1 = nc.dram_tensor("wv1", [64, 64], BF16, kind="ExternalInput")
    bias = nc.dram_tensor("bias", [128, 3], F32, kind="ExternalInput")
    out = nc.dram_tensor("out", [BP, C, H, W], BF16, kind="ExternalOutput")

    xpd_ap = xpd.ap()
    out_ap = out.ap()

    with tile.TileContext(nc) as tc, ExitStack() as ctx:
        const_pool = ctx.enter_context(tc.tile_pool(name="const", bufs=1))
        x1_pool = ctx.enter_context(tc.tile_pool(name="x1", bufs=4))
        x2_pool = ctx.enter_context(tc.tile_pool(name="x2", bufs=4))
        xe_pool = ctx.enter_context(tc.tile_pool(name="xe", bufs=2))
        q_pool = ctx.enter_context(tc.tile_pool(name="qsb", bufs=2))
        vt_pool = ctx.enter_context(tc.tile_pool(name="vt", bufs=3))
        prod_pool = ctx.enter_context(tc.tile_pool(name="prod", bufs=3))
        red_pool = ctx.enter_context(tc.tile_pool(name="red", bufs=2))
        outp_pool = ctx.enter_context(tc.tile_pool(name="outp", bufs=2))
        psum_pool = ctx.enter_context(tc.tile_pool(name="psum", bufs=2, space="PSUM"))
        psv_pool = psum_pool

        wqkp_sb = const_pool.tile([128, 3, 2, 128], BF16)
        wqk2_sb = const_pool.tile([128, 2, 128], BF16)
        wqk1_sb = const_pool.tile([64, 2, 128], BF16)
        wvp_sb = const_pool.tile([128, 3, 64], BF16)
        wv2_sb = const_pool.tile([128, 64], BF16)
        wv1_sb = const_pool.tile([64, 64], BF16)
        b_sb = const_pool.tile([128, 3], F32)

        def _load_consts():
            nc.sync.dma_start(wqkp_sb[:], wqkp.ap())
            nc.sync.dma_start(wqk2_sb[:], wqk2.ap())
            nc.sync.dma_start(wqk1_sb[:], wqk1.ap())
            nc.sync.dma_start(wvp_sb[:], wvp.ap())
            nc.sync.dma_start(wv2_sb[:], wv2.ap())
            nc.sync.dma_start(wv1_sb[:], wv1.ap())
            nc.sync.dma_start(b_sb[:], bias.ap())

        def _body():
          # Hoist all image DMAs: HWDGE queues issue in order, so putting
          # both pairs' loads up front lets pair p+1's images stream while
          # pair p computes.
          # Per-sample images.  X1: lower half = padded rows 0..65, upper
          # half = padded rows 1..66 (one-row shift -> tap pairs
          # (0,c)+(1,c)).  X2: lower half = padded rows 2..66 (tap (2,c)),
          # upper half additionally shifted one col (tap (2,c+1)).
          # Tiles keep the full DRAM row width (WPD) so every load is one
          # contiguous run per partition; matmul APs slice columns for free.
          imgs, xes = [], []
          for p in range(PAIRS):
            x1_t, x2_t = [], []
            for s in range(2):
                src = xpd_ap[2 * p + s]
                src_flat = src.rearrange("c h w -> c (h w)")
                t1 = x1_pool.tile([128, X1ROWS, WPD], BF16, tag="x1")
                nc.sync.dma_start(t1[0:64], src[:, 0:X1ROWS, :])
                nc.scalar.dma_start(t1[64:128], src[:, 1:X1ROWS + 1, :])
                x1_t.append(t1)
                t2 = x2_pool.tile([128, X2ROWS, WPD], BF16, tag="x2")
                nc.sync.dma_start(t2[0:64], src[:, 2:2 + X2ROWS, :])
                # one-col shift: contiguous flat run starting at (2,1)
                nc.scalar.dma_start(
                    t2[64:128],
                    src_flat[:, 2 * WPD + 1: 2 * WPD + 1 + X2ROWS * WPD]
                    .rearrange("c (h w) -> c h w", w=WPD),
                )
                x2_t.append(t2)
            imgs.append((x1_t[0], x1_t[1], x2_t[0], x2_t[1]))
            # residual x (bf16) in pair layout [A|B]; full-width rows so the
            # load is contiguous, interior selected in the consuming AP
            xe = xe_pool.tile([128, H, WPD], BF16)
            nc.gpsimd.dma_start(
                xe[:],
                xpd_ap[2 * p:2 * p + 2, :, 1:H + 1, :]
                .rearrange("b c h w -> (b c) h w"),
            )
            xes.append(xe)

          for p in range(PAIRS):
            X1A, X1B, X2A, X2B = imgs[p]
            xe = xes[p]

            q_sb = q_pool.tile([128, NCH, NF], F32)
            kvp = red_pool.tile([128, NCH], F32, tag="kvp")
            for j in range(NCH):
                rb = RJ * j
                psA = psum_pool.tile([128, NF], F32, tag="psA")
                psB = psum_pool.tile([128, NF], F32, tag="psB")
                # vA and vB share one bank: [vB | vA].  Each stream starts
                # its own partition range (has_written clears are
                # per-partition); skip_group_check silences the sim's
                # one-group-per-bank bookkeeping.
                psV = psv_pool.tile([128, NF], F32, tag="psV")
                mm = nc.tensor.matmul

                # qk matmuls, sample A then B: 3 X1 tap-pairs (K=128) +
                # 1 X2 tap-pair (K=128) + 1 X2 single (K=64), all rhs
                # base-partition 0, M=128 = [q|k] (A) / [k|q] (B)
                for v, X1, X2, ps in ((0, X1A, X2A, psA), (1, X1B, X2B, psB)):
                    for c in range(3):
                        mm(ps[:], wqkp_sb[:, c, v, :],
                           X1[:, rb:rb + RJ, c:c + W],
                           start=(c == 0), stop=False)
                    mm(ps[:], wqk2_sb[:, v, :],
                       X2[:, rb:rb + RJ, 0:W], start=False, stop=False)
                    mm(ps[:], wqk1_sb[:, v, :],
                       X2[0:64, rb:rb + RJ, 2:2 + W], start=False, stop=True)
                # v matmuls: same taps, M=64; A -> psVA[64:], B -> psVB[:64],
                # interleaved for col-group concurrency
                for c in range(3):
                    mm(psV[64:128], wvp_sb[:, c, :], X1A[:, rb:rb + RJ, c:c + W],
                       start=(c == 0), stop=False, skip_group_check=True)
                    mm(psV[0:64], wvp_sb[:, c, :], X1B[:, rb:rb + RJ, c:c + W],
                       start=(c == 0), stop=False, skip_group_check=True)
                mm(psV[64:128], wv2_sb[:], X2A[:, rb:rb + RJ, 0:W],
                   start=False, stop=False, skip_group_check=True)
                mm(psV[0:64], wv2_sb[:], X2B[:, rb:rb + RJ, 0:W],
                   start=False, stop=False, skip_group_check=True)
                mm(psV[64:128], wv1_sb[:], X2A[0:64, rb:rb + RJ, 2:2 + W],
                   start=False, stop=False, skip_group_check=True)
                mm(psV[0:64], wv1_sb[:], X2B[0:64, rb:rb + RJ, 2:2 + W],
                   start=False, stop=True, skip_group_check=True)

                # drains: q with fused bias (ScalarE), v with fused bias
                nc.scalar.activation(
                    q_sb[0:64, j, :], psA[0:64], AF.Identity,
                    bias=b_sb[0:64, 0:1])
                nc.scalar.activation(
                    q_sb[64:128, j, :], psB[64:128], AF.Identity,
                    bias=b_sb[64:128, 0:1])
                vt = vt_pool.tile([128, NF], F32, tag="vt")
                nc.scalar.activation(
                    vt[:], psV[:], AF.Identity, bias=b_sb[:, 2:3])
                # kv partial: (k~ + bk) * v summed over the chunk (DVE),
                # k~ read straight from PSUM
                prod = prod_pool.tile([128, NF], F32, tag="prod")
                nc.vector.scalar_tensor_tensor(
                    out=prod[64:128],
                    in0=psA[64:128],
                    scalar=b_sb[64:128, 1:2],
                    in1=vt[64:128],
                    op0=ALU.add,
                    op1=ALU.mult,
                    accum_out=kvp[64:128, j:j + 1],
                )
                nc.vector.scalar_tensor_tensor(
                    out=prod[0:64],
                    in0=psB[0:64],
                    scalar=b_sb[0:64, 1:2],
                    in1=vt[0:64],
                    op0=ALU.add,
                    op1=ALU.mult,
                    accum_out=kvp[0:64, j:j + 1],
                )

            # total kv per channel; kv = [kvB | kvA] -> swap to [kvA | kvB]
            kv = red_pool.tile([128, 1], F32, tag="kv")
            nc.vector.tensor_reduce(
                kv[:], kvp[:], axis=mybir.AxisListType.X, op=ALU.add
            )
            kvsw = red_pool.tile([128, 1], F32, tag="kvsw")
            # on the SWDGE ring so the image HWDGE rings stay pure prefetch
            nc.gpsimd.dma_start(kvsw[0:64], kv[64:128])
            nc.gpsimd.dma_start(kvsw[64:128], kv[0:64])

            # out = q * kv + x, one DVE op per chunk
            o_sb = outp_pool.tile([128, NCH, NF], BF16)
            for j in range(NCH):
                nc.vector.scalar_tensor_tensor(
                    out=o_sb[:, j, :].rearrange("p (a b) -> p a b", a=RJ),
                    in0=q_sb[:, j, :].rearrange("p (a b) -> p a b", a=RJ),
                    scalar=kvsw[:, 0:1],
                    in1=xe[:, RJ * j:RJ * j + RJ, 1:1 + W],
                    op0=ALU.mult,
                    op1=ALU.add,
                )
            nc.gpsimd.dma_start(
                out_ap[2 * p:2 * p + 2],
                o_sb[:],
            )

        if reps == 1:
            _load_consts()
            _body()
        else:
            from concourse.engine_type import EngineType

            _load_consts()
            with tc.For_i(0, reps, 1, hint_engines=(EngineType.PE,),
                          staggered_reset=True):
                _body()

    nc.compile()
    return nc


def _get_nc(reps=1):
    if reps not in _NC_CACHE:
        _NC_CACHE[reps] = _build_nc(reps)
    return _NC_CACHE[reps]


def _pack_weights(wq, bq, wk, bk, wv, bv, gamma):
    g = float(np.asarray(gamma, np.float32).reshape(-1)[0])
    wqg = np.asarray(wq, np.float32) * g
    wkf = np.asarray(wk, np.float32)
    wvf = np.asarray(wv, np.float32)
    bqg = np.asarray(bq, np.float32) * g
    bkf = np.asarray(bk, np.float32)
    bvf = np.asarray(bv, np.float32)

    def t(w, dy, dx):
        # lhsT block [in_ch, out_ch] for one tap
        return w[:, :, dy, dx].T

    # conv column order per sample variant: A = [q|k], B = [k|q]
    pairs_a = ((wqg, 0), (wkf, 64))
    pairs_b = ((wkf, 0), (wqg, 64))

    wqkp = np.zeros((128, 3, 2, 128), np.float32)
    for c in range(3):
        for v, cols in ((0, pairs_a), (1, pairs_b)):
            for w, o in cols:
                wqkp[0:64, c, v, o:o + 64] = t(w, 0, c)
                wqkp[64:128, c, v, o:o + 64] = t(w, 1, c)

    wqk2 = np.zeros((128, 2, 128), np.float32)
    wqk1 = np.zeros((64, 2, 128), np.float32)
    for v, cols in ((0, pairs_a), (1, pairs_b)):
        for w, o in cols:
            wqk2[0:64, v, o:o + 64] = t(w, 2, 0)
            wqk2[64:128, v, o:o + 64] = t(w, 2, 1)
            wqk1[:, v, o:o + 64] = t(w, 2, 2)

    wvp = np.zeros((128, 3, 64), np.float32)
    for c in range(3):
        wvp[0:64, c, :] = t(wvf, 0, c)
        wvp[64:128, c, :] = t(wvf, 1, c)
    wv2 = np.zeros((128, 64), np.float32)
    wv2[0:64, :] = t(wvf, 2, 0)
    wv2[64:128, :] = t(wvf, 2, 1)
    wv1 = t(wvf, 2, 2)

    bias = np.zeros((128, 3), np.float32)
    for c, b in enumerate((bqg, bkf, bvf)):
        bias[0:64, c] = b
        bias[64:128, c] = b

    bf = ml_dtypes.bfloat16
    return (wqkp.astype(bf), wqk2.astype(bf), wqk1.astype(bf),
            wvp.astype(bf), wv2.astype(bf), wv1.astype(bf), bias)


def _pack_inputs(x):
    xp = np.zeros((B, C, HPD, WPD), ml_dtypes.bfloat16)
    xp[:, :, 1:H + 1, 1:W + 1] = x.astype(ml_dtypes.bfloat16)
    return xp


def _in_maps(x, wq, bq, wk, bk, wv, bv, gamma):
    wqkp, wqk2, wqk1, wvp, wv2, wv1, bias = _pack_weights(
        wq, bq, wk, bk, wv, bv, gamma)
    xp = _pack_inputs(np.ascontiguousarray(np.asarray(x, np.float32)))
    return [
        {
            "xpd": xp[BP * i:BP * (i + 1)],
            "wqkp": wqkp,
            "wqk2": wqk2,
            "wqk1": wqk1,
            "wvp": wvp,
            "wv2": wv2,
            "wv1": wv1,
            "bias": bias,
        }
        for i in range(NCORES)
    ]


def kernel(x, wq, bq, wk, bk, wv, bv, gamma):
    x = np.ascontiguousarray(np.asarray(x, np.float32))
    assert x.shape == (B, C, H, W), x.shape
    in_maps = _in_maps(x, wq, bq, wk, bk, wv, bv, gamma)
    nc = _get_nc()
    res = run_bass_kernel_spmd(nc, in_maps, core_ids=list(range(NCORES)))
    global LAST_RESULTS
    LAST_RESULTS = res
    return np.concatenate(
        [np.asarray(res.results[i]["out"], np.float32) for i in range(NCORES)],
        axis=0,
    )


def time_kernel(inputs, reps_lo=512, reps_hi=8192, calls=3):
    """Estimate per-iteration HW exec time by differencing two on-device
    repeat-loop variants (call overhead and transfers cancel)."""
    import time as _time

    in_maps = _in_maps(
        inputs["x"], inputs["wq"], inputs["bq"], inputs["wk"], inputs["bk"],
        inputs["wv"], inputs["bv"], inputs["gamma"],
    )
    nc_lo, nc_hi = _get_nc(reps_lo), _get_nc(reps_hi)
    cores = list(range(NCORES))
    run_bass_kernel_spmd(nc_lo, in_maps, core_ids=cores)
    run_bass_kernel_spmd(nc_hi, in_maps, core_ids=cores)
    deltas = []
    walls = {}
    for _ in range(calls + 2):
        t0 = _time.time()
        run_bass_kernel_spmd(nc_lo, in_maps, core_ids=cores)
        t1 = _time.time()
        run_bass_kernel_spmd(nc_hi, in_maps, core_ids=cores)
        t2 = _time.time()
        walls[reps_lo] = min(walls.get(reps_lo, 1e9), t1 - t0)
        walls[reps_hi] = min(walls.get(reps_hi, 1e9), t2 - t1)
        deltas.append(((t2 - t1) - (t1 - t0)) / (reps_hi - reps_lo) * 1e9)
    deltas.sort()
    return deltas[len(deltas) // 2], walls
